# revision 53
# baseline (speedup 1.0000x reference)
"""AttentionHead kernel for 8 Trainium2 NeuronCores.

Problem (per sample, B=4): x:[256,64,64] -> q/k/v 1x1-conv projections
(+positional encoding on q,k), S = q^T k / 8, softmax over the QUERY axis,
out = attn @ v, then 1x1-conv MLP with Mish + residual.

Sharding: 2 cores per sample, split over the query axis i (2048 queries each).
Softmax normalizes over i, so the per-key denominator den[j] = sum_i exp(S[i,j])
needs one tiny AllReduce per core pair (done in 4 chunks; each chunk's latency
hides behind compute); den then folds into v (v/den), everything else is local,
and the output halves are disjoint.

Layout trick: compute S transposed, S[j,i] = (k^T q)[j,i], keys j on partitions.
exp runs PSUM->SBUF with a per-partition accumulate (the denominator for free),
and exp(S)[j,i] is then directly the correct operand layout for both
out[c,i] = sum_j v[c,j]*attnT[j,i] and the MLP - zero on-device transposes.
All matmul operands bf16 (fp32 PSUM accumulation).

Bias handling: q/k biases are folded into the positional-encoding tensors on
the host; the v bias is a broadcast tensor added during the PSUM->SBUF move;
b1 rides the Mish activation's per-partition bias; b2 rides the residual add.
Mish = x*tanh(softplus(x)): sp via exp+ln on the ScalarE LUTs, then Tanh,
then one DVE multiply; the three activation batches are ordered so the LUT
table set switches only twice.

Input DMAs are merged into a few big transfers (issue cost on the sequencer is
~650ns each) and split across the sync and gpsimd queues.
"""

import numpy as np
import ml_dtypes

import concourse.bass as bass
import concourse.bacc as bacc
import concourse.mybir as mybir
import concourse.tile as tile
from concourse.tile_rust import add_dep_helper

BF16 = mybir.dt.bfloat16
F32 = mybir.dt.float32
AF = mybir.ActivationFunctionType
OP = mybir.AluOpType
bf16 = ml_dtypes.bfloat16

B, C, H, W = 4, 256, 64, 64
N = H * W            # 4096 pixels
QK = 64
IS = N // 2          # 2048 queries per core
NJT = N // 128       # 32 key tiles
NIB = IS // 512      # 4 i-blocks
NCH = 4              # den allreduce chunks
JCH = NJT // NCH     # 8 key tiles per chunk
N_CORES = 8
REPLICA_GROUPS = [[0, 1], [2, 3], [4, 5], [6, 7]]


def build_program(n_cores: int = N_CORES, enable_asserts: bool = False) -> bass.Bass:
    nc = bacc.Bacc(
        "TRN2",
        target_bir_lowering=False,
        debug=False,
        enable_asserts=enable_asserts,
        num_devices=n_cores,
    )

    # Per-core inputs (data differs by core; program is identical).
    # xq/xb/xf hold the two 128-row channel halves side by side:
    # [:, kt*COLS : (kt+1)*COLS] is channel rows kt*128..kt*128+127.
    xq_d = nc.dram_tensor("xq", [128, 2 * IS], BF16, kind="ExternalInput").ap()
    xb_d = nc.dram_tensor("xb", [128, 2 * N], BF16, kind="ExternalInput").ap()
    xf_d = nc.dram_tensor("xf", [128, 2 * IS], F32, kind="ExternalInput").ap()
    pe1q_d = nc.dram_tensor("pe1q", [QK, IS], BF16, kind="ExternalInput").ap()
    # Shared weights (same on all cores).
    pe1_d = nc.dram_tensor("pe1", [QK, N], BF16, kind="ExternalInput").ap()
    wqk_d = nc.dram_tensor("wqk", [128, 256], BF16, kind="ExternalInput").ap()
    # wmlp = wvt | w1t | w2t | bvb
    wmlp_d = nc.dram_tensor("wmlp", [128, 1792], BF16, kind="ExternalInput").ap()
    bcols_d = nc.dram_tensor("bcols", [128, 4], F32, kind="ExternalInput").ap()

    y_d = nc.dram_tensor("y", [C, IS], F32, kind="ExternalOutput").ap()

    with tile.TileContext(nc) as tc:
        with (
            tc.tile_pool(name="const", bufs=1) as cpool,
            tc.tile_pool(name="qk", bufs=1) as qkpool,
            tc.tile_pool(name="outsb", bufs=1) as outpool,
            tc.tile_pool(name="den", bufs=1) as denpool,
            tc.tile_pool(name="dram", bufs=1, space="DRAM") as dram,
        ):
            # q-projection inputs first (gate the whole pipeline), then the
            # rest; bulky non-urgent loads go on the gpsimd queue.
            wqk_sb = cpool.tile([128, 256], BF16)
            nc.sync.dma_start(wqk_sb[:], wqk_d[:])

            q_sb = qkpool.tile([QK, IS], BF16)     # q, d on partitions
            k_sb = qkpool.tile([QK, N], BF16)      # k, d on partitions
            vtpool = tc.alloc_tile_pool(name="vt", bufs=1, side="right")
            vt_sb = vtpool.tile([128, NJT * 256], BF16)   # v^T, j on partitions
            den_sb = denpool.tile([128, NJT], F32)
            dsum_sb = denpool.tile([128, NJT], F32)
            rden_sb = denpool.tile([128, NJT], F32)
            out_sb = [outpool.tile([128, IS], BF16, name=f"out_sb{i}") for i in range(2)]

            # ---- Phase 1 + early S: projections interleaved with the first
            # NBOOT S-tiles (computed in [128,1024] PSUM halves so they fit
            # alongside the projection PSUM pools) to start ScalarE's exp
            # stream as early as possible. ----
            NBOOT = 10
            with tc.tile_pool(name="attn", bufs=1) as apool:
                attn_sb = apool.tile([128, NJT * IS], BF16)   # 16 MiB
                with (
                    tc.tile_pool(name="xq", bufs=1) as xqpool,
                    tc.tile_pool(name="xb", bufs=1) as xbpool,
                    tc.tile_pool(name="pe", bufs=1) as pepool,
                    tc.tile_pool(name="psA", bufs=2, space="PSUM") as psA,
                    tc.tile_pool(name="psV", bufs=2, space="PSUM") as psV,
                    tc.tile_pool(name="psS0", bufs=2, space="PSUM") as psS0,
                ):
                    xq_sb = xqpool.tile([128, 2 * IS], BF16)
                    xb_sb = xbpool.tile([128, 2 * N], BF16)
                    pe1q_sb = pepool.tile([QK, IS], BF16)
                    pe1_sb = pepool.tile([QK, N], BF16)
                    # DMA order follows the consumption chain: q inputs,
                    # then BOTH channel-half xb chunks + pe1 half that
                    # k_proj(0-3)/v_proj(0-15) need, then the rest.
                    # k-side inputs (xb halves 0/2 + pe1 half 0) stream on
                    # the gpsimd queue in parallel with the q-side loads on
                    # sync, so the k-projection chain isn't serialized behind
                    # the q bytes.
                    nc.sync.dma_start(xq_sb[:, bass.ts(0, IS)],
                                      xq_d[:, bass.ts(0, IS)])
                    nc.sync.dma_start(pe1q_sb[:], pe1q_d[:])
                    nc.gpsimd.dma_start(xb_sb[:, bass.ts(0, N // 2)],
                                        xb_d[:, bass.ts(0, N // 2)])
                    nc.gpsimd.dma_start(xb_sb[:, bass.ts(2, N // 2)],
                                        xb_d[:, bass.ts(2, N // 2)])
                    nc.gpsimd.dma_start(pe1_sb[:, bass.ts(0, N // 2)],
                                        pe1_d[:, bass.ts(0, N // 2)])
                    nc.sync.dma_start(xq_sb[:, bass.ts(1, IS)],
                                      xq_d[:, bass.ts(1, IS)])
                    nc.sync.dma_start(xb_sb[:, bass.ts(1, N // 2)],
                                      xb_d[:, bass.ts(1, N // 2)])
                    nc.sync.dma_start(xb_sb[:, bass.ts(3, N // 2)],
                                      xb_d[:, bass.ts(3, N // 2)])
                    nc.sync.dma_start(pe1_sb[:, bass.ts(1, N // 2)],
                                      pe1_d[:, bass.ts(1, N // 2)])
                    wmlp_sb = cpool.tile([128, 1792], BF16)
                    bcols_sb = cpool.tile([128, 4], F32)
                    nc.gpsimd.dma_start(wmlp_sb[:], wmlp_d[:])
                    nc.gpsimd.dma_start(bcols_sb[:], bcols_d[:])
                    wvt = wmlp_sb[:, 0:512]
                    w1t = wmlp_sb[:, 512:1024]
                    w2t = wmlp_sb[:, 1024:1536]
                    bvb = wmlp_sb[:, 1536:1792]
                    b1c = bcols_sb[:, 0:2]
                    b2c = bcols_sb[:, 2:4]

                    den_h = denpool.tile([128, 2 * NBOOT], F32)

                    def q_proj(ib, pool=None, pname="psa"):
                        sl = bass.ts(ib, 512)
                        ps = (pool or psA).tile([QK, 512], F32, name=pname)
                        for kt in range(2):
                            nc.tensor.matmul(ps[:], wqk_sb[:, bass.ts(kt, QK)],
                                             xq_sb[:, ib * 1024 + kt * 512:
                                                   ib * 1024 + (kt + 1) * 512],
                                             start=(kt == 0), stop=(kt == 1))
                        nc.vector.tensor_add(q_sb[:, sl], ps[:], pe1q_sb[:, sl])

                    def k_proj(jb):
                        sl = bass.ts(jb, 512)
                        ps = psA.tile([QK, 512], F32, name="psa")
                        for kt in range(2):
                            nc.tensor.matmul(ps[:], wqk_sb[:, 128 + kt * QK:
                                                           128 + (kt + 1) * QK],
                                             xb_sb[:, kt * N + jb * 512:
                                                   kt * N + (jb + 1) * 512],
                                             start=(kt == 0), stop=(kt == 1))
                        nc.vector.tensor_add(k_sb[:, sl], ps[:], pe1_sb[:, sl])

                    def s_boot(jt):
                        for h2 in range(2):
                            ps0 = psS0.tile([128, 1024], F32, name="pss0")
                            for n2 in range(2):
                                ib = h2 * 2 + n2
                                nc.tensor.matmul(ps0[:, bass.ts(n2, 512)],
                                                 k_sb[:, bass.ts(jt, 128)],
                                                 q_sb[:, bass.ts(ib, 512)],
                                                 start=True, stop=True)
                            nc.scalar.activation(
                                attn_sb[:, jt * IS + h2 * 1024:
                                        jt * IS + (h2 + 1) * 1024],
                                ps0[:], AF.Exp, scale=0.125,
                                accum_out=den_h[:, h2 * NBOOT + jt:
                                                h2 * NBOOT + jt + 1])

                    def v_proj(jt):
                        ps = psV.tile([128, 256], F32, name="psv")
                        for kt in range(2):
                            nc.tensor.matmul(ps[:],
                                             xb_sb[:, kt * N + jt * 128:
                                                   kt * N + (jt + 1) * 128],
                                             wvt[:, bass.ts(kt, 256)],
                                             start=(kt == 0), stop=(kt == 1))
                        nc.vector.tensor_add(vt_sb[:, bass.ts(jt, 256)],
                                             ps[:], bvb[:])

                    # k0/k1 ahead of q2/q3: the in-order DVE queue must
                    # reach the k adds before the adds that wait on the late
                    # xq chunk; q2's matmul then waits k0's evacuation, but
                    # s_boot needed k0 at that moment anyway.
                    q_proj(0)
                    q_proj(1)
                    k_proj(0)
                    k_proj(1)
                    # q2/q3 psums ride the (still idle) psV ring so their
                    # matmuls don't serialize on k0's evacuation in the
                    # 2-slot psA ring.
                    q_proj(2, pool=psV, pname="psv")
                    q_proj(3, pool=psV, pname="psv")
                    for jt in range(4):
                        s_boot(jt)
                    k_proj(2)
                    for jt in range(4, 8):
                        s_boot(jt)
                    k_proj(3)
                    for jt in range(8, NBOOT):
                        s_boot(jt)
                    for jb in range(4, N // 512):
                        k_proj(jb)
                    for jt in range(NJT):
                        v_proj(jt)
                    nc.vector.tensor_add(den_sb[:, 0:NBOOT],
                                         den_h[:, 0:NBOOT],
                                         den_h[:, NBOOT:2 * NBOOT])

                # ---- Phase 2: remaining S[j,i] = (k^T q)/8, attnT = exp(S) ----
                # den for these key tiles comes from a DVE row-reduce over the
                # bf16 attn tile instead of the activation accumulator: the
                # READ_ACCUMULATOR op costs ~290ns each on the pacing ScalarE
                # queue, while the DVE has slack here.  The den AllReduces,
                # reciprocals and v^T folds are interleaved into the loop
                # (chunk c's allreduce fires as soon as its last tile is
                # reduced; the reciprocal+folds three tiles later, once the
                # ~8us collective has landed, so the DVE never head-of-line
                # blocks on it).
                vtspool = tc.alloc_tile_pool(name="vts", bufs=1)
                vts_sb = vtspool.tile([128, NJT * 256], BF16)  # v^T / den

                def den_ar(ch):
                    csl = bass.ts(ch, JCH)
                    den_in = dram.tile([128, JCH], F32, name=f"den_in{ch}")
                    den_out = dram.tile([128, JCH], F32, name=f"den_out{ch}")
                    nc.sync.dma_start(den_in[:], den_sb[:, csl])
                    nc.gpsimd.collective_compute(
                        "AllReduce", OP.add,
                        replica_groups=REPLICA_GROUPS,
                        ins=[den_in.opt()], outs=[den_out.opt()],
                    )
                    nc.sync.dma_start(dsum_sb[:, csl], den_out[:])

                def den_fold(ch):
                    csl = bass.ts(ch, JCH)
                    nc.vector.reciprocal(rden_sb[:, csl], dsum_sb[:, csl])
                    for jt in range(ch * JCH, (ch + 1) * JCH):
                        nc.vector.tensor_scalar_mul(vts_sb[:, bass.ts(jt, 256)],
                                                    vt_sb[:, bass.ts(jt, 256)],
                                                    rden_sb[:, jt:jt + 1])

                with tc.tile_pool(name="psS", bufs=2, space="PSUM") as psS:
                    for jt in range(NBOOT, NJT):
                        if jt == NBOOT:
                            den_ar(0)   # key tiles 0-7: den ready from boot
                        ps = psS.tile([128, IS], F32)
                        for ib in range(NIB):
                            nc.tensor.matmul(ps[:, bass.ts(ib, 512)],
                                             k_sb[:, bass.ts(jt, 128)],
                                             q_sb[:, bass.ts(ib, 512)],
                                             start=True, stop=True)
                        nc.scalar.activation(attn_sb[:, bass.ts(jt, IS)], ps[:],
                                             AF.Exp, scale=0.125)
                        nc.vector.tensor_reduce(
                            den_sb[:, jt:jt + 1], attn_sb[:, bass.ts(jt, IS)],
                            axis=mybir.AxisListType.XYZW, op=OP.add)
                        if jt >= 2 * JCH - 1 and (jt + 1) % JCH == 0:
                            den_ar(jt // JCH)
                        if jt % JCH == 5 and jt // JCH < NCH - 1:
                            den_fold(jt // JCH - 1)
                for ch in range(NCH - 2, NCH):
                    den_fold(ch)
                vtpool.release()

                # ---- Phase 3: out[c,i] = sum_j vts[j,c] * attnT[j,i] ----
                # Two j-half visits so the last den chunk's allreduce hides
                # behind the first visits' matmuls.
                # Phase 4's matmuls reuse the 8 out-accumulator PSUM banks
                # (Tile serializes on the read->overwrite dependencies), and
                # visit 2 runs ib-major so each i-block's MLP front (W1 matmul
                # + mish on ScalarE/DVE) overlaps the remaining attn@v work.
                with (
                    tc.tile_pool(name="xf", bufs=1) as xfpool,
                    tc.tile_pool(name="h", bufs=1) as hpool,
                    tc.tile_pool(name="mtmp", bufs=2) as mpool,
                    tc.tile_pool(name="y", bufs=2) as ypool,
                    tc.tile_pool(name="psO", bufs=1, space="PSUM") as psO,
                ):
                    xf_sb = xfpool.tile([128, 2 * IS], F32)
                    nc.gpsimd.dma_start(xf_sb[:], xf_d[:])
                    h_sb = [hpool.tile([128, IS], BF16, name=f"h_sb{i}")
                            for i in range(2)]
                    pso = {}
                    for mt in range(2):
                        for ib in range(NIB):
                            pso[mt, ib] = psO.tile([128, 512], F32,
                                                   name=f"pso{mt}{ib}")
                    def av_mms(mt, ib, jlo, jhi):
                        for jt in range(jlo, jhi):
                            nc.tensor.matmul(
                                pso[mt, ib][:],
                                vts_sb[:, jt * 256 + mt * 128:
                                       jt * 256 + (mt + 1) * 128],
                                attn_sb[:, jt * IS + ib * 512:
                                        jt * IS + (ib + 1) * 512],
                                start=(jt == 0), stop=(jt == NJT - 1),
                                skip_group_check=True)

                    def w2_y(ib):
                        sl = bass.ts(ib, 512)
                        for mt in range(2):
                            ps = pso[mt, ib]
                            for kt in range(2):
                                nc.tensor.matmul(
                                    ps[:],
                                    w2t[:, kt * 256 + mt * 128:
                                        kt * 256 + (mt + 1) * 128],
                                    h_sb[kt][:, sl],
                                    start=(kt == 0), stop=(kt == 1),
                                    skip_group_check=True)
                            y_sb = ypool.tile([128, 512], F32)
                            nc.vector.scalar_tensor_tensor(
                                y_sb[:], ps[:], b2c[:, mt:mt + 1],
                                xf_sb[:, mt * IS + ib * 512:
                                      mt * IS + (ib + 1) * 512],
                                op0=OP.add, op1=OP.add)
                            nc.sync.dma_start(
                                y_d[mt * 128:(mt + 1) * 128, sl], y_sb[:])

                    for v in range(NCH - 1):
                        for mt in range(2):
                            for ib in range(NIB):
                                av_mms(mt, ib, v * JCH, (v + 1) * JCH)
                    # Visit 2 runs ib-major; each i-block's MLP chases its
                    # final attn@v chunk, and W2(ib-1) is pipelined between
                    # ib's attn@v and W1 so the PE never waits on the mish
                    # chain.  mish = (x)*(1 - 2/((1+e^x)^2+1)): ScalarE Exp +
                    # Square (both in the already-loaded LUT set - ZERO table
                    # switches) + a 4-op DVE chain with the table-free fast
                    # approximate reciprocal.  The out copies ride ScalarE
                    # Identity so the DVE queue never gates W1.
                    for ib in range(NIB):
                        sl = bass.ts(ib, 512)
                        for mt in range(2):
                            av_mms(mt, ib, (NCH - 1) * JCH, NJT)
                        if ib > 0:
                            w2_y(ib - 1)
                        for mt in range(2):
                            nc.scalar.activation(out_sb[mt][:, sl],
                                                 pso[mt, ib][:], AF.Identity)
                        for mt in range(2):
                            ps = pso[mt, ib]
                            for kt in range(2):
                                nc.tensor.matmul(
                                    ps[:],
                                    w1t[:, kt * 256 + mt * 128:
                                        kt * 256 + (mt + 1) * 128],
                                    out_sb[kt][:, sl],
                                    start=(kt == 0), stop=(kt == 1),
                                    skip_group_check=True)
                            e_t = mpool.tile([128, 512], BF16, name="mish_e")
                            u_t = mpool.tile([128, 512], F32, name="mish_u")
                            r_t = mpool.tile([128, 512], F32, name="mish_r")
                            nc.scalar.activation(e_t[:], ps[:], AF.Exp,
                                                 bias=b1c[:, mt:mt + 1])
                            nc.scalar.activation(u_t[:], e_t[:], AF.Square,
                                                 bias=1.0)
                            nc.vector.tensor_scalar_add(u_t[:], u_t[:], 1.0)
                            nc.vector.reciprocal_approx_fast(r_t[:], u_t[:])
                            # th = 1 - 2r overwrites the dead e tile
                            nc.scalar.activation(e_t[:], r_t[:],
                                                 AF.Identity,
                                                 bias=1.0, scale=-2.0)
                            nc.vector.scalar_tensor_tensor(
                                h_sb[mt][:, sl], ps[:],
                                b1c[:, mt:mt + 1], e_t[:],
                                op0=OP.add, op1=OP.mult)
                    w2_y(NIB - 1)
                vtspool.release()
    nc.finalize()
    return nc


def _to_lhsT_sb(w):
    """[256, M] fp32 -> SBUF layout [128, 2*M] bf16: col block kt holds rows
    kt*128..kt*128+127 of w."""
    k, m = w.shape
    assert k == 256
    return np.ascontiguousarray(
        w.reshape(2, 128, m).transpose(1, 0, 2).reshape(128, 2 * m).astype(bf16))


def _bf(a):
    return np.ascontiguousarray(np.asarray(a, dtype=np.float32).astype(bf16))


def _halves(a):
    """[256, X] -> [128, 2*X] with the two 128-row halves side by side."""
    return np.ascontiguousarray(np.concatenate([a[:128], a[128:]], axis=1))


def make_in_maps(x, WQ, bQ, WK, bK, WV, bV, PE, W1, b1, W2, b2, n_cores=N_CORES):
    x = np.asarray(x, dtype=np.float32)
    xf3 = np.ascontiguousarray(x.reshape(B, C, N))
    pef = np.asarray(PE, dtype=np.float32).reshape(QK, N)
    pe1 = _bf(pef + np.asarray(bK, np.float32)[:, None])
    pe1q_full = _bf(pef + np.asarray(bQ, np.float32)[:, None])

    wq = _to_lhsT_sb(np.asarray(WQ, np.float32).T)   # [128, 128]
    wk = _to_lhsT_sb(np.asarray(WK, np.float32).T)
    wmlp = np.concatenate([
        _to_lhsT_sb(np.asarray(WV, np.float32).T),
        _to_lhsT_sb(np.asarray(W1, np.float32).T),
        _to_lhsT_sb(np.asarray(W2, np.float32).T),
        np.broadcast_to(_bf(np.asarray(bV)[None, :]), (128, 256)),
    ], axis=1)
    bcols = np.concatenate([
        np.asarray(b1, np.float32).reshape(2, 128).T,
        np.asarray(b2, np.float32).reshape(2, 128).T,
    ], axis=1)

    shared = {
        "pe1": pe1,
        "wqk": np.ascontiguousarray(np.concatenate([wq, wk], axis=1)),
        "wmlp": np.ascontiguousarray(wmlp),
        "bcols": np.ascontiguousarray(bcols),
    }
    in_maps = []
    for core in range(n_cores):
        s, h = core // 2, core % 2
        isl = slice(h * IS, (h + 1) * IS)
        xb = _bf(xf3[s])
        m = dict(shared)
        m["xb"] = _halves(xb)
        # xq is ib-interleaved: [:, ib*1024+kt*512 : ...] = channel-half kt,
        # query block ib - so the first DMA chunk covers ib 0-1 completely.
        xqs = xb[:, isl]
        m["xq"] = np.ascontiguousarray(np.concatenate(
            [np.concatenate([xqs[:128, ib * 512:(ib + 1) * 512],
                             xqs[128:, ib * 512:(ib + 1) * 512]], axis=1)
             for ib in range(NIB)], axis=1))
        m["xf"] = _halves(xf3[s][:, isl])
        m["pe1q"] = np.ascontiguousarray(pe1q_full[:, isl])
        in_maps.append(m)
    return in_maps


def assemble_output(results, n_cores=N_CORES):
    y = np.empty((B, C, N), dtype=np.float32)
    for s in range(B):
        y[s][:, :IS] = results[2 * s]["y"]
        y[s][:, IS:] = results[2 * s + 1]["y"]
    return y.reshape(B, C, H, W)


_PROG = None


def kernel(**inputs) -> np.ndarray:
    global _PROG
    from concourse.bass_utils import run_bass_kernel_spmd
    if _PROG is None:
        _PROG = build_program(N_CORES)
    in_maps = make_in_maps(**inputs)
    res = run_bass_kernel_spmd(_PROG, in_maps, core_ids=list(range(N_CORES)))
    return assemble_output(res.results)
